# revision 34
# baseline (speedup 1.0000x reference)
"""Causal self-attention (B=4, T=2048, C=1024, H=16) on 8 TRN2 NeuronCores.

Sharding: core c handles batch b = c//2 and heads [8*(c%2), 8*(c%2)+8).
Each core computes the qkv projection for its 8 heads, flash-style causal
attention, and a partial output projection (its heads' slice of W_out rows).
Host sums the two partials per batch and adds the bias terms (v/out biases
are affine in the output because softmax rows sum to 1).

Device layouts (per core):
  xT [C, T]      x[b] transposed (host-side transpose)
  qT/kT [512,T]  bf16, heads stacked along partitions (64 rows/head)
  v  [T, 8*65]   bf16 natural, per head 65 cols: [v(64) | ones] -- the ones
                 column makes the AV matmul emit the softmax denominator.
  S^T blocks [128(s), 512(t)] computed in causally-trimmed, column-compacted
                 PSUM pairs -> one wide exp (scale fused) -> bf16 probs^T
  out^T accum [65, 512] in PSUM via the ones column (denominator for free);
                 normalize = batched 1/den = exp(-ln(den)) on ACT (ln shares
                 exp's activation-table set -- see _patch_act_tables) ->
                 gpsimd partition_broadcast -> one DVE multiply.
All matmuls in bf16: uniform dtype keeps the PE out of fp32 mode switches
and enables fast weight loads; fp32 PSUM accumulation throughout.

Schedule (v2): phases are interleaved so the ACT engine (softmax exp, the
attention-phase bottleneck) starts ~15us into the kernel instead of after
the full qkv projection, and the PE stream is software-pipelined
(QK lookahead 2 over the exp->mask->AV chain) so the tensor engine stays
busy and at its max p-state. x is DMA'd in 512-column chunks so the first
v-projection matmuls start ~4us in. out_proj results DMA straight from
PSUM to DRAM (no SBUF staging copy).
"""
from contextlib import ExitStack

import numpy as np
import concourse.bass as bass  # noqa: F401  (registers engines)
import concourse.mybir as mybir
import concourse.tile as tile
from concourse import bacc
from concourse.bass_utils import run_bass_kernel_spmd

# problem constants (hardcoded per contract)
B, T, C, H, D = 4, 2048, 1024, 16, 64
NCORES = 8
NH = H // 2          # heads per core = 8
QK = NH * D          # 512 qkv cols per core per q/k/v
SCALE = float(D) ** -0.5
P = 128
NKT = C // P         # 8 contraction tiles for the projections
NMQ = QK // P        # 4 row-tiles of qT/kT
NTT = T // P         # 16 t-blocks
NQC = T // 512       # 4 q-chunks
FP = mybir.dt.float32
FPR = mybir.dt.float32r
BF = mybir.dt.bfloat16
EXP = mybir.ActivationFunctionType.Exp
LOG = mybir.ActivationFunctionType.Ln

_NC_CACHE = {}
_LAST_IN_MAPS = None


def _patch_act_tables():
    """Force exp and ln onto the one table set that holds both, so the
    ACT engine never thrashes table loads between the softmax exp and the
    ln-based reciprocal. Entries keep their positions (act_func_set_id is
    positional), they just stop advertising exp/ln."""
    import concourse.bacc as _bacc
    import concourse.hw_specs as _hw

    orig = _hw.get_activation_tables

    def patched(arch):
        tables = orig(arch)
        for name, fns in tables.items():
            if name != "natural_log_exp_and_others":
                fns.discard(mybir.ActivationFunctionType.Exp)
                fns.discard(mybir.ActivationFunctionType.Ln)
        return tables

    _bacc.get_activation_tables = patched


_patch_act_tables()


def _pairs_spans(qc):
    """Per q-chunk: the causally-trimmed, column-compacted (t-block pair)
    spans. Each entry: list of (j, lo, dst, w) for the pair's two t-blocks:
    matmul kT[j-block] x q[:, lo:] -> psum[:, dst:dst+w]."""
    jmax = 4 * qc + 3
    npairs = (jmax + 1) // 2
    out = []
    for pr in range(npairs):
        spans = []
        dst = 0
        for i in range(2):
            j = 2 * pr + i
            r = j - 4 * qc
            lo = 0 if r < 0 else 128 * r
            w = 512 - lo
            dst = max(dst, i * 512 if r < 1 else 0)
            spans.append((j, lo, dst, w))
            dst += w
        out.append(spans)
    return out


def build_nc():
    if "nc" in _NC_CACHE:
        return _NC_CACHE["nc"]
    nc = bacc.Bacc(target_bir_lowering=False)
    xT = nc.declare_dram_parameter("xT", [C, T], BF, isOutput=False)
    Wq = nc.declare_dram_parameter("Wq", [NMQ, C, P], BF, isOutput=False)
    Wk = nc.declare_dram_parameter("Wk", [NMQ, C, P], BF, isOutput=False)
    Wv = nc.declare_dram_parameter("Wv", [C, QK], BF, isOutput=False)
    bq = nc.declare_dram_parameter("bq", [QK, 1], FP, isOutput=False)
    bk = nc.declare_dram_parameter("bk", [QK, 1], FP, isOutput=False)
    Wo = nc.declare_dram_parameter("Wo", [QK, C], BF, isOutput=False)
    y = nc.declare_dram_parameter("y", [T, C], FP, isOutput=True)

    with nc.allow_low_precision(reason="fp32r/bf16 attention"), \
         tile.TileContext(nc) as tc, \
         tc.tile_pool(name="persist", bufs=1) as pers, \
         ExitStack() as stk:
        # ---- persistent tiles (one pool, one slot per tag)
        # weights live in a few big tiles so each loads with ONE dma_start
        # (the sync engine issues DMA descriptors serially at ~0.6us per
        # dma_start -- many small weight DMAs gate the whole startup)
        xall = pers.tile([P, NKT * T], BF, name="xall", tag="xall")
        xsb = [xall[:, k * T:(k + 1) * T] for k in range(NKT)]
        wqall = pers.tile([P, NMQ * NKT * P], BF, name="wqall", tag="wqall")
        wqt = [[wqall[:, (m * NKT + k) * P:(m * NKT + k + 1) * P]
                for k in range(NKT)] for m in range(NMQ)]
        wkall = pers.tile([P, NMQ * NKT * P], BF, name="wkall", tag="wkall")
        wkt = [[wkall[:, (m * NKT + k) * P:(m * NKT + k + 1) * P]
                for k in range(NKT)] for m in range(NMQ)]
        wvall = pers.tile([P, NKT * QK], BF, name="wvall", tag="wvall")
        wvs = [wvall[:, k * QK:(k + 1) * QK] for k in range(NKT)]
        woall = pers.tile([P, NMQ * C], BF, name="woall", tag="woall")
        wot = [woall[:, k * C:(k + 1) * C] for k in range(NMQ)]
        qT = [pers.tile([P, T], BF, name=f"qT{m}", tag=f"qT{m}")
              for m in range(NMQ)]
        kT = [pers.tile([P, T], BF, name=f"kT{m}", tag=f"kT{m}")
              for m in range(NMQ)]
        vsb = [pers.tile([P, NH * 65], BF, name=f"v{t}", tag=f"v{t}")
               for t in range(NTT)]
        aT = [pers.tile([P, T], BF, name=f"aT{m}", tag=f"aT{m}")
              for m in range(NMQ)]
        trit = pers.tile([P, P], BF, name="trit", tag="trit")
        ones_t = pers.tile([1, 64], BF, name="ones_t", tag="ones_t")
        bqt = pers.tile([P, NMQ], FP, name="bqt", tag="bqt")
        bkt = pers.tile([P, NMQ], FP, name="bkt", tag="bkt")

        # upper-triangular (incl diag) keep-mask, built on-device
        nc.gpsimd.memset(trit, 0.0)
        nc.gpsimd.affine_select(
            trit, trit, pattern=[[-1, P]], compare_op=mybir.AluOpType.is_gt,
            fill=1.0, base=0, channel_multiplier=1)
        nc.gpsimd.memset(ones_t, 1.0)
        nc.sync.dma_start(bqt, bq.ap().rearrange("(m p) o -> p (m o)", p=P))
        nc.sync.dma_start(bkt, bk.ap().rearrange("(m p) o -> p (m o)", p=P))

        # ---- input DMAs: few instructions (sync engine issues serially at
        # ~0.6us each) but split into ~256-512KB pieces so several DMA
        # queues stream in parallel; emitted in consumption order
        xall_v = xall.rearrange("p (k t) -> p k t", t=T)
        xT_v = xT.ap().rearrange("(k p) t -> p k t", p=P)
        wv_v = wvall.rearrange("p (k c) -> p k c", c=QK)
        Wv_v = Wv.ap().rearrange("(k p) c -> p k c", p=P)
        wq_v = wqall.rearrange("p (g c) -> p g c", c=P)
        Wq_v = Wq.ap().rearrange("m (k p) c -> p (m k) c", p=P)
        wk_v = wkall.rearrange("p (g c) -> p g c", c=P)
        Wk_v = Wk.ap().rearrange("m (k p) c -> p (m k) c", p=P)
        wo_v = woall.rearrange("p (k c) -> p k c", c=C)
        Wo_v = Wo.ap().rearrange("(k p) c -> p k c", p=P)
        for a in range(4):  # x cols 0:512 (v_pair 0/1, qk sub 0)
            nc.sync.dma_start(xall_v[:, 2 * a:2 * a + 2, 0:256],
                              xT_v[:, 2 * a:2 * a + 2, 0:256])
        for a in range(8):  # Wv: v_pair(0) needs all of it
            nc.sync.dma_start(wv_v[:, a:a + 1, :], Wv_v[:, a:a + 1, :])
        for a in range(2):
            nc.sync.dma_start(xall_v[:, 4 * a:4 * a + 4, 256:512],
                              xT_v[:, 4 * a:4 * a + 4, 256:512])
        nc.sync.dma_start(wq_v[:, 0:16, :], Wq_v[:, 0:16, :])  # m=0,1
        nc.sync.dma_start(wk_v[:, 0:16, :], Wk_v[:, 0:16, :])
        for a in range(2):  # x cols 512:1024 (qk half0 sub 1)
            nc.sync.dma_start(xall_v[:, 4 * a:4 * a + 4, 512:1024],
                              xT_v[:, 4 * a:4 * a + 4, 512:1024])
        nc.sync.dma_start(wq_v[:, 16:32, :], Wq_v[:, 16:32, :])  # m=2,3
        nc.sync.dma_start(wk_v[:, 16:32, :], Wk_v[:, 16:32, :])
        for a in range(2):  # x second half (v_pair 2+, qk half 1)
            nc.sync.dma_start(xall_v[:, 4 * a:4 * a + 4, 1024:2048],
                              xT_v[:, 4 * a:4 * a + 4, 1024:2048])
        for a in range(2):
            nc.sync.dma_start(wo_v[:, 2 * a:2 * a + 2, :],
                              Wo_v[:, 2 * a:2 * a + 2, :])

        pp = stk.enter_context(tc.tile_pool(name="psum", bufs=1, space="PSUM"))
        ep = stk.enter_context(tc.tile_pool(name="evict", bufs=1))

        # ================= emission helpers =================
        def v_pair(tp):
            ps = pp.tile([P, 1024], FP, name=f"pv{tp}", tag="wide", bufs=3)
            for i in range(2):
                t = 2 * tp + i
                for k in range(NKT):
                    nc.tensor.matmul(
                        ps[:, i * 512:(i + 1) * 512],
                        xsb[k][:, t * P:(t + 1) * P], wvs[k],
                        start=(k == 0), stop=(k == NKT - 1))
            for i in range(2):
                t = 2 * tp + i
                vdst = vsb[t].rearrange("p (g w) -> p g w", w=65)
                vsrc = ps[:, i * 512:(i + 1) * 512].rearrange(
                    "p (g w) -> p g w", w=64)
                nc.vector.tensor_copy(vdst[:, :, 0:64], vsrc[:, :, :])
                nc.vector.memset(vdst[:, :, 64:65], 1.0)

        def qk_half(which, m, half):
            dst, bias = (qT, bqt) if which == "q" else (kT, bkt)
            wts = wqt[m] if which == "q" else wkt[m]
            ps = pp.tile([P, 1024], FP, name=f"p{which}{m}{half}", tag="wide",
                         bufs=3)
            for sub in range(2):
                c0 = half * 1024 + sub * 512
                for k in range(NKT):
                    nc.tensor.matmul(
                        ps[:, sub * 512:(sub + 1) * 512],
                        wts[k], xsb[k][:, c0:c0 + 512],
                        start=(k == 0), stop=(k == NKT - 1))
            if half == 0:
                # ACT is idle in the early projection phase; Identity
                # shares the pinned exp/ln table set (no table reload)
                nc.scalar.activation(
                    dst[m][:, 0:1024], ps,
                    mybir.ActivationFunctionType.Identity,
                    bias=bias[:, m:m + 1])
            else:
                nc.vector.tensor_scalar_add(
                    dst[m][:, half * 1024:(half + 1) * 1024], ps,
                    bias[:, m:m + 1])

        onums = {}
        dcols = {}

        def attn_group(qc, heads, fillers=()):
            """Attention for a q-chunk for a pair of heads.

            The two heads of a group live on disjoint 64-row PE quadrants
            (head A: partitions 0:64, head B: 64:128 of the same qT/kT
            tile), and the HW runs quadrant-disjoint matmuls concurrently
            -- so the QK matmuls of both heads are emitted adjacently
            (span-interleaved) to pair them up. The exp->mask->AV chain
            trails the QKs by one round (software pipelining), and filler
            emission (projection work) plugs the ACT-gated PE gaps."""
            spans_all = _pairs_spans(qc)
            jmax = 4 * qc + 3
            npairs = len(spans_all)
            po = {}
            ess = {}
            pss = {}
            for h in heads:
                po[h] = pp.tile([P, 512], FP, name=f"po{h}{qc}", tag="acc",
                                bufs=2)

            def emit_qk_round(pr):
                for h in heads:
                    pss[(h, pr)] = pp.tile([P, 1024], FP,
                                           name=f"ps{h}{qc}{pr}",
                                           tag="wide", bufs=3)
                for sp in range(2):
                    j, lo, d, w = spans_all[pr][sp]
                    for h in heads:
                        ht, ho = h // 2, (h % 2) * 64
                        nc.tensor.matmul(
                            pss[(h, pr)][:, d:d + w],
                            kT[ht][ho:ho + 64, j * P:(j + 1) * P],
                            qT[ht][ho:ho + 64,
                                   qc * 512 + lo:(qc + 1) * 512],
                            start=True, stop=True)

            def emit_exp(pr):
                spans = spans_all[pr]
                e0 = spans[0][2]
                e1 = spans[1][2] + spans[1][3]
                for h in heads:
                    ps = pss.pop((h, pr))
                    es = ep.tile([P, 1024], BF, name=f"es{h}{qc}{pr}",
                                 tag="es", bufs=6)
                    nc.scalar.activation(es[:, e0:e1], ps[:, e0:e1], EXP,
                                         scale=SCALE)
                    for j, lo, d, w in spans:
                        if j - 4 * qc >= 0:
                            nc.vector.tensor_mul(
                                es[:, d:d + 128], es[:, d:d + 128], trit)
                    ess[(h, pr)] = es

            def emit_av(pr):
                for h in heads:
                    vlo = 65 * h
                    es = ess.pop((h, pr))
                    for j, lo, d, w in spans_all[pr]:
                        nc.tensor.matmul(
                            po[h][0:65, lo:lo + w],
                            vsb[j][:, vlo:vlo + 65], es[:, d:d + w],
                            start=(j == 0), stop=(j == jmax))

            fill_iter = iter(fillers)
            for pr in range(npairs):
                emit_qk_round(pr)
                emit_exp(pr)
                if pr > 0:
                    emit_av(pr - 1)
                f = next(fill_iter, None)
                if f is not None:
                    f()
            emit_av(npairs - 1)
            for f in fill_iter:
                f()

            # evict numerator; den row into the 32-stride collector
            for h in heads:
                onum = ep.tile([64, 512], BF, name=f"on{h}{qc}", tag="onum",
                               bufs=9)
                nc.vector.tensor_copy(onum, po[h][0:64, :])
                onums[(qc, h)] = onum
                hp = 32 * (h % 4)
                nc.vector.tensor_copy(dcols[qc][h // 4][hp:hp + 1, :],
                                      po[h][64:65, :])

        def den_half(qc, i, pe_bcast=False):
            """1/den = exp(-ln(den)) on ACT for one 4-head collector;
            broadcast over the 64 head rows (gpsimd, or a rank-1 PE matmul
            when the PE is idle at the kernel tail); one DVE multiply into
            aT. Emitted per collector half so heads 0-3 normalize while
            heads 4-7 are still accumulating."""
            dcol = dcols[qc][i]
            nc.scalar.activation(dcol, dcol, LOG)
            if not pe_bcast:
                dcolr = ep.tile([P, 512], FP, name=f"dcr{qc}_{i}",
                                tag=f"dcolr{i}", bufs=2)
                nc.scalar.activation(dcolr, dcol, EXP, scale=-1.0)
            for h in range(4 * i, 4 * i + 4):
                ht, ho = h // 2, (h % 2) * 64
                hp = 32 * (h % 4)
                if pe_bcast:
                    # per-head recip row at partition 0 (bf16, the rank-1
                    # broadcast matmul's moving operand), ACT is idle here
                    dr = ep.tile([1, 512], BF, name=f"drb{h}{qc}",
                                 tag="rsepb", bufs=2)
                    nc.scalar.activation(dr, dcol[hp:hp + 1, :], EXP,
                                         scale=-1.0)
                    bps = pp.tile([P, 512], FP, name=f"bps{h}{qc}",
                                  tag="acc", bufs=2)
                    nc.tensor.matmul(bps[0:64, :], ones_t, dr,
                                     start=True, stop=True)
                    nc.vector.tensor_mul(
                        aT[ht][ho:ho + 64, qc * 512:(qc + 1) * 512],
                        onums.pop((qc, h)), bps[0:64, :])
                else:
                    rsep = ep.tile([1, 512], FP, name=f"rs{h}{qc}",
                                   tag="rsep", bufs=3)
                    nc.vector.tensor_copy(rsep, dcolr[hp:hp + 1, :])
                    bcs = ep.tile([64, 512], FP, name=f"bc{h}{qc}",
                                  tag="bcs", bufs=4)
                    nc.gpsimd.partition_broadcast(bcs, rsep)
                    nc.vector.tensor_mul(
                        aT[ht][ho:ho + 64, qc * 512:(qc + 1) * 512],
                        onums.pop((qc, h)), bcs)

        proj_ps = {}

        def out_proj_ks(t, ks):
            """Emit the k-contraction subset `ks` of output tile t; on the
            final k the PSUM is staged to SBUF and DMA'd out."""
            if t not in proj_ps:
                proj_ps[t] = pp.tile([P, 1024], FP, name=f"py{t}",
                                     tag="wide", bufs=3)
            ps = proj_ps[t]
            for k in ks:
                for n in range(2):
                    nc.tensor.matmul(
                        ps[:, n * 512:(n + 1) * 512],
                        aT[k][:, t * P:(t + 1) * P],
                        wot[k][:, n * 512:(n + 1) * 512],
                        start=(k == 0), stop=(k == NMQ - 1))
            if ks[-1] == NMQ - 1:
                del proj_ps[t]
                ye = ep.tile([P, 1024], FP, name=f"ye{t}", tag="ye", bufs=3)
                nc.vector.tensor_copy(ye, ps)
                nc.sync.dma_start(y.ap()[t * P:(t + 1) * P, :], ye)

        def out_proj_tile(t):
            out_proj_ks(t, list(range(NMQ)))

        def new_dcols(qc):
            dcols[qc] = [ep.tile([P, 512], FP, name=f"dc{qc}_{i}",
                                 tag=f"dcol{i}", bufs=2) for i in range(2)]

        # ================= emission schedule =================
        v_pair(0)
        v_pair(1)
        new_dcols(0)
        for m in range(NMQ):
            qk_half("q", m, 0)
            qk_half("k", m, 0)
            attn_group(0, (2 * m, 2 * m + 1))
            if m == 1:
                den_half(0, 0)
        den_half(0, 1)
        v_pair(2)
        v_pair(3)
        new_dcols(1)
        attn_group(1, (0, 1), [lambda: qk_half("q", 0, 1)])
        attn_group(1, (2, 3), [lambda: qk_half("k", 0, 1)])
        attn_group(1, (4, 5), [lambda: qk_half("q", 1, 1),
                               lambda: den_half(1, 0)])
        attn_group(1, (6, 7), [lambda: qk_half("k", 1, 1)])
        den_half(1, 1)
        qk_half("q", 2, 1); qk_half("k", 2, 1)
        qk_half("q", 3, 1); qk_half("k", 3, 1)
        v_pair(4)
        v_pair(5)
        new_dcols(2)
        attn_group(2, (0, 1), [lambda: out_proj_tile(0),
                               lambda: v_pair(6)])
        attn_group(2, (2, 3), [lambda: out_proj_tile(1),
                               lambda: v_pair(7)])
        attn_group(2, (4, 5), [lambda: out_proj_tile(2),
                               lambda: den_half(2, 0)])
        attn_group(2, (6, 7), [lambda: out_proj_tile(3)])
        new_dcols(3)
        attn_group(3, (0, 1), [lambda: out_proj_tile(4),
                               lambda: den_half(2, 1),
                               lambda: out_proj_tile(5)])
        attn_group(3, (2, 3), [lambda: out_proj_tile(6),
                               lambda: out_proj_tile(7)])
        attn_group(3, (4, 5), [lambda: out_proj_tile(8),
                               lambda: den_half(3, 0),
                               lambda: out_proj_tile(9)])
        attn_group(3, (6, 7), [lambda: out_proj_tile(10),
                               lambda: out_proj_tile(11)])
        # tail: heads 0-3 of qc3 are already normalized, so the first half
        # of the k-contraction for tiles 12-15 runs while heads 4-7 finish
        # their 1/den (exp on ACT -> rank-1 PE broadcast -> DVE multiply)
        out_proj_ks(12, [0, 1])
        out_proj_ks(13, [0, 1])
        den_half(3, 1, pe_bcast=True)
        out_proj_ks(12, [2, 3])
        out_proj_ks(14, [0, 1])
        out_proj_ks(13, [2, 3])
        out_proj_ks(15, [0, 1])
        out_proj_ks(14, [2, 3])
        out_proj_ks(15, [2, 3])

    nc.compile()
    _NC_CACHE["nc"] = nc
    return nc


def kernel(x, W_qkv, b_qkv, W_out, b_out):
    global _LAST_IN_MAPS
    x = np.asarray(x, dtype=np.float32)
    W_qkv = np.asarray(W_qkv, dtype=np.float32)
    b_qkv = np.asarray(b_qkv, dtype=np.float32)
    W_out = np.asarray(W_out, dtype=np.float32)
    b_out = np.asarray(b_out, dtype=np.float32)
    import ml_dtypes

    bf16 = ml_dtypes.bfloat16
    in_maps = []
    for c in range(NCORES):
        b, hg = c // 2, c % 2
        cols = slice(hg * QK, (hg + 1) * QK)
        wq = W_qkv[:, 0 * C:1 * C][:, cols]
        wk = W_qkv[:, 1 * C:2 * C][:, cols]
        wv = W_qkv[:, 2 * C:3 * C][:, cols]
        in_maps.append({
            "xT": np.ascontiguousarray(x[b].T).astype(bf16),
            "Wq": np.ascontiguousarray(
                wq.reshape(C, NMQ, P).transpose(1, 0, 2)).astype(bf16),
            "Wk": np.ascontiguousarray(
                wk.reshape(C, NMQ, P).transpose(1, 0, 2)).astype(bf16),
            "Wv": np.ascontiguousarray(wv).astype(bf16),
            "bq": np.ascontiguousarray(b_qkv[0 * C:1 * C][cols, None]),
            "bk": np.ascontiguousarray(b_qkv[1 * C:2 * C][cols, None]),
            "Wo": np.ascontiguousarray(W_out[hg * QK:(hg + 1) * QK, :]).astype(bf16),
        })
    _LAST_IN_MAPS = in_maps
    nc = build_nc()
    res = run_bass_kernel_spmd(nc, in_maps, core_ids=list(range(NCORES)))
    # v-bias and output bias are affine in the output: softmax rows sum to 1.
    extra = b_qkv[2 * C:3 * C] @ W_out + b_out
    out = np.empty((B, T, C), dtype=np.float32)
    for b in range(B):
        out[b] = res.results[2 * b]["y"] + res.results[2 * b + 1]["y"] + extra
    return out


# revision 38
# speedup vs baseline: 1.0081x; 1.0081x over previous
"""Causal self-attention (B=4, T=2048, C=1024, H=16) on 8 TRN2 NeuronCores.

Sharding: core c handles batch b = c//2 and heads [8*(c%2), 8*(c%2)+8).
Each core computes the qkv projection for its 8 heads, flash-style causal
attention, and a partial output projection (its heads' slice of W_out rows).
Host sums the two partials per batch and adds the bias terms (v/out biases
are affine in the output because softmax rows sum to 1).

Device layouts (per core):
  xT [C, T]      x[b] transposed (host-side transpose)
  qT/kT [512,T]  bf16, heads stacked along partitions (64 rows/head)
  v  [T, 8*65]   bf16 natural, per head 65 cols: [v(64) | ones] -- the ones
                 column makes the AV matmul emit the softmax denominator.
  S^T blocks [128(s), 512(t)] computed in causally-trimmed, column-compacted
                 PSUM pairs -> one wide exp (scale fused) -> bf16 probs^T
  out^T accum [65, 512] in PSUM via the ones column (denominator for free);
                 normalize = batched 1/den = exp(-ln(den)) on ACT (ln shares
                 exp's activation-table set -- see _patch_act_tables) ->
                 gpsimd partition_broadcast -> one DVE multiply.
All matmuls in bf16: uniform dtype keeps the PE out of fp32 mode switches
and enables fast weight loads; fp32 PSUM accumulation throughout.

Schedule (v2): phases are interleaved so the ACT engine (softmax exp, the
attention-phase bottleneck) starts ~15us into the kernel instead of after
the full qkv projection, and the PE stream is software-pipelined
(QK lookahead 2 over the exp->mask->AV chain) so the tensor engine stays
busy and at its max p-state. x is DMA'd in 512-column chunks so the first
v-projection matmuls start ~4us in. out_proj results DMA straight from
PSUM to DRAM (no SBUF staging copy).
"""
from contextlib import ExitStack

import numpy as np
import concourse.bass as bass  # noqa: F401  (registers engines)
import concourse.mybir as mybir
import concourse.tile as tile
from concourse import bacc
from concourse.bass_utils import run_bass_kernel_spmd

# problem constants (hardcoded per contract)
B, T, C, H, D = 4, 2048, 1024, 16, 64
NCORES = 8
NH = H // 2          # heads per core = 8
QK = NH * D          # 512 qkv cols per core per q/k/v
SCALE = float(D) ** -0.5
P = 128
NKT = C // P         # 8 contraction tiles for the projections
NMQ = QK // P        # 4 row-tiles of qT/kT
NTT = T // P         # 16 t-blocks
NQC = T // 512       # 4 q-chunks
FP = mybir.dt.float32
FPR = mybir.dt.float32r
BF = mybir.dt.bfloat16
EXP = mybir.ActivationFunctionType.Exp
LOG = mybir.ActivationFunctionType.Ln

_NC_CACHE = {}
_LAST_IN_MAPS = None


def _patch_act_tables():
    """Force exp and ln onto the one table set that holds both, so the
    ACT engine never thrashes table loads between the softmax exp and the
    ln-based reciprocal. Entries keep their positions (act_func_set_id is
    positional), they just stop advertising exp/ln."""
    import concourse.bacc as _bacc
    import concourse.hw_specs as _hw

    orig = _hw.get_activation_tables

    def patched(arch):
        tables = orig(arch)
        for name, fns in tables.items():
            if name != "natural_log_exp_and_others":
                fns.discard(mybir.ActivationFunctionType.Exp)
                fns.discard(mybir.ActivationFunctionType.Ln)
        return tables

    _bacc.get_activation_tables = patched


_patch_act_tables()


def _pairs_spans(qc):
    """Per q-chunk: the causally-trimmed, column-compacted (t-block pair)
    spans. Each entry: list of (j, lo, dst, w) for the pair's two t-blocks:
    matmul kT[j-block] x q[:, lo:] -> psum[:, dst:dst+w]."""
    jmax = 4 * qc + 3
    npairs = (jmax + 1) // 2
    out = []
    for pr in range(npairs):
        spans = []
        dst = 0
        for i in range(2):
            j = 2 * pr + i
            r = j - 4 * qc
            lo = 0 if r < 0 else 128 * r
            w = 512 - lo
            dst = max(dst, i * 512 if r < 1 else 0)
            spans.append((j, lo, dst, w))
            dst += w
        out.append(spans)
    return out


def build_nc():
    if "nc" in _NC_CACHE:
        return _NC_CACHE["nc"]
    nc = bacc.Bacc(target_bir_lowering=False)
    xT = nc.declare_dram_parameter("xT", [C, T], BF, isOutput=False)
    Wq = nc.declare_dram_parameter("Wq", [NMQ, C, P], BF, isOutput=False)
    Wk = nc.declare_dram_parameter("Wk", [NMQ, C, P], BF, isOutput=False)
    Wv = nc.declare_dram_parameter("Wv", [C, QK], BF, isOutput=False)
    bq = nc.declare_dram_parameter("bq", [QK, 1], FP, isOutput=False)
    bk = nc.declare_dram_parameter("bk", [QK, 1], FP, isOutput=False)
    Wo = nc.declare_dram_parameter("Wo", [QK, C], BF, isOutput=False)
    y = nc.declare_dram_parameter("y", [T, C], FP, isOutput=True)

    with nc.allow_low_precision(reason="fp32r/bf16 attention"), \
         tile.TileContext(nc) as tc, \
         tc.tile_pool(name="persist", bufs=1) as pers, \
         ExitStack() as stk:
        # ---- persistent tiles (one pool, one slot per tag)
        # weights live in a few big tiles so each loads with ONE dma_start
        # (the sync engine issues DMA descriptors serially at ~0.6us per
        # dma_start -- many small weight DMAs gate the whole startup)
        xall = pers.tile([P, NKT * T], BF, name="xall", tag="xall")
        xsb = [xall[:, k * T:(k + 1) * T] for k in range(NKT)]
        wqall = pers.tile([P, NMQ * NKT * P], BF, name="wqall", tag="wqall")
        wqt = [[wqall[:, (m * NKT + k) * P:(m * NKT + k + 1) * P]
                for k in range(NKT)] for m in range(NMQ)]
        wkall = pers.tile([P, NMQ * NKT * P], BF, name="wkall", tag="wkall")
        wkt = [[wkall[:, (m * NKT + k) * P:(m * NKT + k + 1) * P]
                for k in range(NKT)] for m in range(NMQ)]
        wvall = pers.tile([P, NKT * QK], BF, name="wvall", tag="wvall")
        wvs = [wvall[:, k * QK:(k + 1) * QK] for k in range(NKT)]
        woall = pers.tile([P, NMQ * C], BF, name="woall", tag="woall")
        wot = [woall[:, k * C:(k + 1) * C] for k in range(NMQ)]
        qT = [pers.tile([P, T], BF, name=f"qT{m}", tag=f"qT{m}")
              for m in range(NMQ)]
        kT = [pers.tile([P, T], BF, name=f"kT{m}", tag=f"kT{m}")
              for m in range(NMQ)]
        vsb = [pers.tile([P, NH * 65], BF, name=f"v{t}", tag=f"v{t}")
               for t in range(NTT)]
        aT = [pers.tile([P, T], BF, name=f"aT{m}", tag=f"aT{m}")
              for m in range(NMQ)]
        trit = pers.tile([P, P], BF, name="trit", tag="trit")
        ones_t = pers.tile([1, 64], BF, name="ones_t", tag="ones_t")
        bqt = pers.tile([P, NMQ], FP, name="bqt", tag="bqt")
        bkt = pers.tile([P, NMQ], FP, name="bkt", tag="bkt")

        # upper-triangular (incl diag) keep-mask, built on-device
        nc.gpsimd.memset(trit, 0.0)
        nc.gpsimd.affine_select(
            trit, trit, pattern=[[-1, P]], compare_op=mybir.AluOpType.is_gt,
            fill=1.0, base=0, channel_multiplier=1)
        nc.gpsimd.memset(ones_t, 1.0)
        nc.sync.dma_start(bqt, bq.ap().rearrange("(m p) o -> p (m o)", p=P))
        nc.sync.dma_start(bkt, bk.ap().rearrange("(m p) o -> p (m o)", p=P))

        # ---- input DMAs: few instructions (sync engine issues serially at
        # ~0.6us each) but split into ~256-512KB pieces so several DMA
        # queues stream in parallel; emitted in consumption order
        xall_v = xall.rearrange("p (k t) -> p k t", t=T)
        xT_v = xT.ap().rearrange("(k p) t -> p k t", p=P)
        wv_v = wvall.rearrange("p (k c) -> p k c", c=QK)
        Wv_v = Wv.ap().rearrange("(k p) c -> p k c", p=P)
        wq_v = wqall.rearrange("p (g c) -> p g c", c=P)
        Wq_v = Wq.ap().rearrange("m (k p) c -> p (m k) c", p=P)
        wk_v = wkall.rearrange("p (g c) -> p g c", c=P)
        Wk_v = Wk.ap().rearrange("m (k p) c -> p (m k) c", p=P)
        wo_v = woall.rearrange("p (k c) -> p k c", c=C)
        Wo_v = Wo.ap().rearrange("(k p) c -> p k c", p=P)
        for a in range(4):  # x cols 0:512 (v_pair 0/1, qk sub 0)
            nc.sync.dma_start(xall_v[:, 2 * a:2 * a + 2, 0:256],
                              xT_v[:, 2 * a:2 * a + 2, 0:256])
        for a in range(8):  # Wv: v_pair(0) needs all of it
            nc.sync.dma_start(wv_v[:, a:a + 1, :], Wv_v[:, a:a + 1, :])
        for a in range(2):
            nc.sync.dma_start(xall_v[:, 4 * a:4 * a + 4, 256:512],
                              xT_v[:, 4 * a:4 * a + 4, 256:512])
        nc.sync.dma_start(wq_v[:, 0:16, :], Wq_v[:, 0:16, :])  # m=0,1
        nc.sync.dma_start(wk_v[:, 0:16, :], Wk_v[:, 0:16, :])
        for a in range(2):  # x cols 512:1024 (qk half0 sub 1)
            nc.sync.dma_start(xall_v[:, 4 * a:4 * a + 4, 512:1024],
                              xT_v[:, 4 * a:4 * a + 4, 512:1024])
        nc.sync.dma_start(wq_v[:, 16:32, :], Wq_v[:, 16:32, :])  # m=2,3
        nc.sync.dma_start(wk_v[:, 16:32, :], Wk_v[:, 16:32, :])
        for a in range(2):  # x second half (v_pair 2+, qk half 1)
            nc.sync.dma_start(xall_v[:, 4 * a:4 * a + 4, 1024:2048],
                              xT_v[:, 4 * a:4 * a + 4, 1024:2048])
        for a in range(2):
            nc.sync.dma_start(wo_v[:, 2 * a:2 * a + 2, :],
                              Wo_v[:, 2 * a:2 * a + 2, :])

        pp = stk.enter_context(tc.tile_pool(name="psum", bufs=1, space="PSUM"))
        ep = stk.enter_context(tc.tile_pool(name="evict", bufs=1))

        # ================= emission helpers =================
        def v_pair(tp):
            ps = pp.tile([P, 1024], FP, name=f"pv{tp}", tag="wide", bufs=3)
            for i in range(2):
                t = 2 * tp + i
                for k in range(NKT):
                    nc.tensor.matmul(
                        ps[:, i * 512:(i + 1) * 512],
                        xsb[k][:, t * P:(t + 1) * P], wvs[k],
                        start=(k == 0), stop=(k == NKT - 1))
            for i in range(2):
                t = 2 * tp + i
                vdst = vsb[t].rearrange("p (g w) -> p g w", w=65)
                vsrc = ps[:, i * 512:(i + 1) * 512].rearrange(
                    "p (g w) -> p g w", w=64)
                nc.vector.tensor_copy(vdst[:, :, 0:64], vsrc[:, :, :])
                nc.vector.memset(vdst[:, :, 64:65], 1.0)

        def qk_half(which, m, half):
            dst, bias = (qT, bqt) if which == "q" else (kT, bkt)
            wts = wqt[m] if which == "q" else wkt[m]
            ps = pp.tile([P, 1024], FP, name=f"p{which}{m}{half}", tag="wide",
                         bufs=3)
            for sub in range(2):
                c0 = half * 1024 + sub * 512
                for k in range(NKT):
                    nc.tensor.matmul(
                        ps[:, sub * 512:(sub + 1) * 512],
                        wts[k], xsb[k][:, c0:c0 + 512],
                        start=(k == 0), stop=(k == NKT - 1))
            if half == 0:
                # ACT is idle in the early projection phase; Identity
                # shares the pinned exp/ln table set (no table reload)
                nc.scalar.activation(
                    dst[m][:, 0:1024], ps,
                    mybir.ActivationFunctionType.Identity,
                    bias=bias[:, m:m + 1])
            else:
                nc.vector.tensor_scalar_add(
                    dst[m][:, half * 1024:(half + 1) * 1024], ps,
                    bias[:, m:m + 1])

        onums = {}
        dcols = {}

        def attn_group(qc, heads, fillers=()):
            """Attention for a q-chunk for a pair of heads.

            The two heads of a group live on disjoint 64-row PE quadrants
            (head A: partitions 0:64, head B: 64:128 of the same qT/kT
            tile), and the HW runs quadrant-disjoint matmuls concurrently
            -- so the QK matmuls of both heads are emitted adjacently
            (span-interleaved) to pair them up. The exp->mask->AV chain
            trails the QKs by one round (software pipelining), and filler
            emission (projection work) plugs the ACT-gated PE gaps."""
            spans_all = _pairs_spans(qc)
            jmax = 4 * qc + 3
            npairs = len(spans_all)
            po = {}
            ess = {}
            pss = {}
            for h in heads:
                po[h] = pp.tile([P, 512], FP, name=f"po{h}{qc}", tag="acc",
                                bufs=2)

            def emit_qk_round(pr):
                for h in heads:
                    pss[(h, pr)] = pp.tile([P, 1024], FP,
                                           name=f"ps{h}{qc}{pr}",
                                           tag="wide", bufs=3)
                for sp in range(2):
                    j, lo, d, w = spans_all[pr][sp]
                    for h in heads:
                        ht, ho = h // 2, (h % 2) * 64
                        nc.tensor.matmul(
                            pss[(h, pr)][:, d:d + w],
                            kT[ht][ho:ho + 64, j * P:(j + 1) * P],
                            qT[ht][ho:ho + 64,
                                   qc * 512 + lo:(qc + 1) * 512],
                            start=True, stop=True)

            def emit_exp(pr):
                spans = spans_all[pr]
                e0 = spans[0][2]
                e1 = spans[1][2] + spans[1][3]
                for h in heads:
                    ps = pss.pop((h, pr))
                    es = ep.tile([P, 1024], BF, name=f"es{h}{qc}{pr}",
                                 tag="es", bufs=6)
                    nc.scalar.activation(es[:, e0:e1], ps[:, e0:e1], EXP,
                                         scale=SCALE)
                    for j, lo, d, w in spans:
                        if j - 4 * qc >= 0:
                            nc.vector.tensor_mul(
                                es[:, d:d + 128], es[:, d:d + 128], trit)
                    ess[(h, pr)] = es

            def emit_av(pr):
                for h in heads:
                    vlo = 65 * h
                    es = ess.pop((h, pr))
                    for j, lo, d, w in spans_all[pr]:
                        nc.tensor.matmul(
                            po[h][0:65, lo:lo + w],
                            vsb[j][:, vlo:vlo + 65], es[:, d:d + w],
                            start=(j == 0), stop=(j == jmax))

            fill_iter = iter(fillers)
            for pr in range(npairs):
                emit_qk_round(pr)
                emit_exp(pr)
                if pr > 0:
                    emit_av(pr - 1)
                f = next(fill_iter, None)
                if f is not None:
                    f()
            emit_av(npairs - 1)
            for f in fill_iter:
                f()

            # evict numerator; den row into the 32-stride collector
            for h in heads:
                onum = ep.tile([64, 512], BF, name=f"on{h}{qc}", tag="onum",
                               bufs=9)
                nc.vector.tensor_copy(onum, po[h][0:64, :])
                onums[(qc, h)] = onum
                hp = 32 * (h % 4)
                nc.vector.tensor_copy(dcols[qc][h // 4][hp:hp + 1, :],
                                      po[h][64:65, :])

        def den_half(qc, i, pe_bcast=False):
            """1/den = exp(-ln(den)) on ACT for one 4-head collector;
            broadcast over the 64 head rows (gpsimd, or a rank-1 PE matmul
            when the PE is idle at the kernel tail); one DVE multiply into
            aT. Emitted per collector half so heads 0-3 normalize while
            heads 4-7 are still accumulating."""
            dcol = dcols[qc][i]
            nc.scalar.activation(dcol, dcol, LOG)
            if not pe_bcast:
                dcolr = ep.tile([P, 512], FP, name=f"dcr{qc}_{i}",
                                tag=f"dcolr{i}", bufs=2)
                nc.scalar.activation(dcolr, dcol, EXP, scale=-1.0)
            for h in range(4 * i, 4 * i + 4):
                ht, ho = h // 2, (h % 2) * 64
                hp = 32 * (h % 4)
                if pe_bcast:
                    # per-head recip row at partition 0 (bf16, the rank-1
                    # broadcast matmul's moving operand), ACT is idle here
                    dr = ep.tile([1, 512], BF, name=f"drb{h}{qc}",
                                 tag="rsepb", bufs=2)
                    nc.scalar.activation(dr, dcol[hp:hp + 1, :], EXP,
                                         scale=-1.0)
                    bps = pp.tile([P, 512], FP, name=f"bps{h}{qc}",
                                  tag="acc", bufs=2)
                    nc.tensor.matmul(bps[0:64, :], ones_t, dr,
                                     start=True, stop=True)
                    nc.vector.tensor_mul(
                        aT[ht][ho:ho + 64, qc * 512:(qc + 1) * 512],
                        onums.pop((qc, h)), bps[0:64, :])
                else:
                    rsep = ep.tile([1, 512], FP, name=f"rs{h}{qc}",
                                   tag="rsep", bufs=3)
                    nc.vector.tensor_copy(rsep, dcolr[hp:hp + 1, :])
                    bcs = ep.tile([64, 512], FP, name=f"bc{h}{qc}",
                                  tag="bcs", bufs=4)
                    nc.gpsimd.partition_broadcast(bcs, rsep)
                    nc.vector.tensor_mul(
                        aT[ht][ho:ho + 64, qc * 512:(qc + 1) * 512],
                        onums.pop((qc, h)), bcs)

        proj_ps = {}

        def out_proj_ks(t, ks):
            """Emit the k-contraction subset `ks` of output tile t; on the
            final k the PSUM is staged to SBUF and DMA'd out."""
            if t not in proj_ps:
                proj_ps[t] = pp.tile([P, 1024], FP, name=f"py{t}",
                                     tag="wide", bufs=3)
            ps = proj_ps[t]
            for k in ks:
                for n in range(2):
                    nc.tensor.matmul(
                        ps[:, n * 512:(n + 1) * 512],
                        aT[k][:, t * P:(t + 1) * P],
                        wot[k][:, n * 512:(n + 1) * 512],
                        start=(k == 0), stop=(k == NMQ - 1))
            if ks[-1] == NMQ - 1:
                del proj_ps[t]
                ye = ep.tile([P, 1024], FP, name=f"ye{t}", tag="ye", bufs=3)
                nc.vector.tensor_copy(ye, ps)
                nc.sync.dma_start(y.ap()[t * P:(t + 1) * P, :], ye)

        def out_proj_tile(t):
            out_proj_ks(t, list(range(NMQ)))

        def new_dcols(qc):
            dcols[qc] = [ep.tile([P, 512], FP, name=f"dc{qc}_{i}",
                                 tag=f"dcol{i}", bufs=2) for i in range(2)]

        # ================= emission schedule =================
        v_pair(0)
        v_pair(1)
        new_dcols(0)
        for m in range(NMQ):
            qk_half("q", m, 0)
            qk_half("k", m, 0)
            attn_group(0, (2 * m, 2 * m + 1))
            if m == 1:
                den_half(0, 0)
        den_half(0, 1)
        v_pair(2)
        v_pair(3)
        new_dcols(1)
        attn_group(1, (0, 1), [lambda: qk_half("q", 0, 1)])
        attn_group(1, (2, 3), [lambda: qk_half("k", 0, 1)])
        den_half(1, 0)
        attn_group(1, (4, 5), [lambda: qk_half("q", 1, 1)])
        attn_group(1, (6, 7), [lambda: qk_half("k", 1, 1)])
        den_half(1, 1)
        qk_half("q", 2, 1); qk_half("k", 2, 1)
        qk_half("q", 3, 1); qk_half("k", 3, 1)
        v_pair(4)
        v_pair(5)
        new_dcols(2)
        attn_group(2, (0, 1), [lambda: out_proj_tile(0),
                               lambda: v_pair(6)])
        attn_group(2, (2, 3), [lambda: out_proj_tile(1),
                               lambda: v_pair(7)])
        den_half(2, 0)
        attn_group(2, (4, 5), [lambda: out_proj_tile(2)])
        attn_group(2, (6, 7), [lambda: out_proj_tile(3)])
        den_half(2, 1)
        new_dcols(3)
        attn_group(3, (0, 1), [lambda: out_proj_tile(4),
                               lambda: out_proj_tile(5)])
        attn_group(3, (2, 3), [lambda: out_proj_tile(6),
                               lambda: out_proj_tile(7)])
        den_half(3, 0)
        attn_group(3, (4, 5), [lambda: out_proj_tile(8),
                               lambda: out_proj_tile(9)])
        attn_group(3, (6, 7), [lambda: out_proj_tile(10),
                               lambda: out_proj_tile(11)])
        den_half(3, 1, pe_bcast=True)
        for t in range(12, NTT):
            out_proj_tile(t)

    nc.compile()
    _NC_CACHE["nc"] = nc
    return nc


def kernel(x, W_qkv, b_qkv, W_out, b_out):
    global _LAST_IN_MAPS
    x = np.asarray(x, dtype=np.float32)
    W_qkv = np.asarray(W_qkv, dtype=np.float32)
    b_qkv = np.asarray(b_qkv, dtype=np.float32)
    W_out = np.asarray(W_out, dtype=np.float32)
    b_out = np.asarray(b_out, dtype=np.float32)
    import ml_dtypes

    bf16 = ml_dtypes.bfloat16
    in_maps = []
    for c in range(NCORES):
        b, hg = c // 2, c % 2
        cols = slice(hg * QK, (hg + 1) * QK)
        wq = W_qkv[:, 0 * C:1 * C][:, cols]
        wk = W_qkv[:, 1 * C:2 * C][:, cols]
        wv = W_qkv[:, 2 * C:3 * C][:, cols]
        in_maps.append({
            "xT": np.ascontiguousarray(x[b].T).astype(bf16),
            "Wq": np.ascontiguousarray(
                wq.reshape(C, NMQ, P).transpose(1, 0, 2)).astype(bf16),
            "Wk": np.ascontiguousarray(
                wk.reshape(C, NMQ, P).transpose(1, 0, 2)).astype(bf16),
            "Wv": np.ascontiguousarray(wv).astype(bf16),
            "bq": np.ascontiguousarray(b_qkv[0 * C:1 * C][cols, None]),
            "bk": np.ascontiguousarray(b_qkv[1 * C:2 * C][cols, None]),
            "Wo": np.ascontiguousarray(W_out[hg * QK:(hg + 1) * QK, :]).astype(bf16),
        })
    _LAST_IN_MAPS = in_maps
    nc = build_nc()
    res = run_bass_kernel_spmd(nc, in_maps, core_ids=list(range(NCORES)))
    # v-bias and output bias are affine in the output: softmax rows sum to 1.
    extra = b_qkv[2 * C:3 * C] @ W_out + b_out
    out = np.empty((B, T, C), dtype=np.float32)
    for b in range(B):
        out[b] = res.results[2 * b]["y"] + res.results[2 * b + 1]["y"] + extra
    return out
